# revision 1
# baseline (speedup 1.0000x reference)
"""Trainium2 Bass kernel for CoreProcessor (soft memory-slot routing).

Computation (per token t):
    q = x Wq^T + bq
    a = softmax((q keys^T) / sqrt(d))
    out = sum_m a[m] * (ops[m] @ x)

Sharding: data-parallel over the 16384 tokens across 8 cores (2048 each);
keys/ops/Wq/bq replicated.

Per-core structure:
  - Fold the query projection into the logits:  logits = x @ KWT + bl
    with KWT = Wq^T keys^T / sqrt(d)  [128,64],  bl = bq keys^T / sqrt(d) [1,64]
    (both computed on-device once on the PE).
  - ops[m] pre-transposed on the PE into opsT [e=128, (m,d)=8192].
  - Per 128-token tile: PE-transpose x -> xT; logits via 2 matmuls (bias
    added with a K=1 ones-matmul); softmax with the running sum fused into
    the ACT exp via accum_out; p normalized by 1/Z up-front.
  - Main: 16 matmuls [128tok x 512] (float32r, 4 slots per chunk) into PSUM.
    Each chunk is drained two ways in parallel: DVE scalar_tensor_tensor FMA
    granules into two interleaved SBUF accumulators, and ACT scaled copies
    (bf16) summed by cheap PE identity-matmuls into a PSUM accumulator
    (GPSIMD cannot read PSUM, so it is not used for draining).
"""

import sys

import numpy as np

sys.path.insert(0, "/opt/trn_rl_repo")

import concourse.bass as bass  # noqa: E402
import concourse.tile as tile  # noqa: E402
from concourse import bacc, mybir  # noqa: E402
from concourse.bass_utils import run_bass_kernel_spmd  # noqa: E402
from concourse.masks import make_identity  # noqa: E402

F32 = mybir.dt.float32
F32R = mybir.dt.float32r

N_CORES = 8
B, S, D, M = 4, 4096, 128, 64
NTOK_TOTAL = B * S           # 16384
NTOK = NTOK_TOTAL // N_CORES  # 2048 tokens per core
NT = NTOK // 128              # 16 token tiles per core
NCHUNK = (M * D) // 512       # 16 rhs chunks of 512 (4 slots each)
SCALE = 1.0 / float(np.sqrt(np.float32(D)))

# Drain split: within each 4-slot chunk, how many granules go to DVE
# (the rest go to the ACT scaled-copy + PE identity-matmul path).
DVE_PER_CHUNK = 2

_CACHE = {}


def _build(y_dtype=F32R, dve_per_chunk=DVE_PER_CHUNK):
    from contextlib import ExitStack

    nc = bacc.Bacc("TRN2", target_bir_lowering=False, debug=False)

    x_d = nc.dram_tensor("x", [NTOK, D], F32, kind="ExternalInput")
    keys_d = nc.dram_tensor("keys", [M, D], F32, kind="ExternalInput")
    ops_d = nc.dram_tensor("ops", [M, D, D], F32, kind="ExternalInput")
    wq_d = nc.dram_tensor("wq", [D, D], F32, kind="ExternalInput")
    bq_d = nc.dram_tensor("bq", [D], F32, kind="ExternalInput")
    out_d = nc.dram_tensor("out", [NTOK, D], F32, kind="ExternalOutput")

    with tile.TileContext(nc) as tc, ExitStack() as ctx:
        consts = ctx.enter_context(tc.tile_pool(name="consts", bufs=1))
        stage = ctx.enter_context(tc.tile_pool(name="stage", bufs=3))
        xt_pool = ctx.enter_context(tc.tile_pool(name="xt", bufs=4))
        p_pool = ctx.enter_context(tc.tile_pool(name="p", bufs=3))
        small = ctx.enter_context(tc.tile_pool(name="small", bufs=6))
        acc_pool = ctx.enter_context(tc.tile_pool(name="acc", bufs=4))
        out_pool = ctx.enter_context(tc.tile_pool(name="outp", bufs=3))
        tr_ps = ctx.enter_context(
            tc.tile_pool(name="trps", bufs=2, space=bass.MemorySpace.PSUM)
        )
        l_ps = tr_ps
        y_ps = ctx.enter_context(
            tc.tile_pool(name="yps", bufs=4, space=bass.MemorySpace.PSUM)
        )
        pa_ps = ctx.enter_context(
            tc.tile_pool(name="paps", bufs=2, space=bass.MemorySpace.PSUM)
        )
        z_pool = ctx.enter_context(tc.tile_pool(name="z", bufs=8))

        # ---- constants ----
        ident = consts.tile([128, 128], F32)
        make_identity(nc, ident)
        ones_row = consts.tile([1, 128], F32)
        nc.vector.memset(ones_row, 1.0)

        keys_sb = consts.tile([M, D], F32)
        nc.sync.dma_start(keys_sb, keys_d[:])
        wq_sb = consts.tile([D, D], F32)
        nc.sync.dma_start(wq_sb, wq_d[:])
        bq_sb = consts.tile([D, 1], F32)
        nc.sync.dma_start(bq_sb, bq_d.rearrange("(p o) -> p o", o=1))

        # all of x for this core: [t%128, tile, e]
        x_sb = consts.tile([128, NT, D], F32)
        nc.sync.dma_start(x_sb, x_d.rearrange("(n p) e -> p n e", p=128))

        # keysT [e, m]
        ktp = tr_ps.tile([D, M], F32, tag="tr")
        nc.tensor.transpose(ktp, keys_sb, ident[:M, :M])
        keysT_sb = consts.tile([D, M], F32)
        nc.scalar.copy(keysT_sb, ktp)

        # KWT = Wq^T keys^T / sqrt(d):  kwt[d, m] = sum_e Wq[e,d] keysT[e,m]
        kwtp = tr_ps.tile([D, M], F32, tag="tr")
        nc.tensor.matmul(kwtp, wq_sb, keysT_sb, start=True, stop=True)
        kwt_sb = consts.tile([D, M], y_dtype)
        nc.scalar.mul(kwt_sb, kwtp, SCALE)

        # bl = bq keys^T / sqrt(d):  [1, m]
        blp = l_ps.tile([128, M], F32, tag="tr")
        nc.tensor.matmul(blp[:1, :], bq_sb, keysT_sb, start=True, stop=True)
        bl_sb = consts.tile([1, M], F32)
        nc.scalar.mul(bl_sb, blp[:1, :], SCALE)

        # bf16 identity for the cheap PE accumulate-matmuls
        ident_bf = consts.tile([128, 128], mybir.dt.bfloat16)
        nc.vector.tensor_copy(ident_bf, ident)

        # opsT [e, (m,d)]: PE-transpose each ops[m]
        opsT_sb = consts.tile([D, M * D], y_dtype)
        copy_engines = [nc.scalar, nc.vector]
        for m in range(M):
            om = stage.tile([D, D], F32, tag="opsm")
            nc.sync.dma_start(om, ops_d[m])
            otp = tr_ps.tile([D, D], F32, tag="tr")
            nc.tensor.transpose(otp, om, ident)
            eng = copy_engines[m % 2]
            if eng is nc.scalar:
                eng.copy(opsT_sb[:, m * D:(m + 1) * D], otp)
            else:
                eng.tensor_copy(opsT_sb[:, m * D:(m + 1) * D], otp)

        # ---- main loop over token tiles ----
        for i in range(NT):
            # xT [e, t]
            xtp = tr_ps.tile([128, 128], F32, tag="tr")
            nc.tensor.transpose(xtp, x_sb[:, i, :], ident)
            xT = xt_pool.tile([128, 128], y_dtype)
            nc.scalar.copy(xT, xtp)

            # logits [t, m] = x @ KWT + bl   (pre-scaled by 1/sqrt(d))
            lp = l_ps.tile([128, M], F32, tag="tr")
            nc.tensor.matmul(lp, xT, kwt_sb, start=True, stop=False)
            nc.tensor.matmul(lp, ones_row, bl_sb, start=False, stop=True)

            # softmax (unnormalized exp, then fold 1/Z into p)
            nmx = small.tile([128, 1], F32, tag="nmx")
            nc.vector.tensor_reduce(
                nmx, lp, axis=mybir.AxisListType.X, op=mybir.AluOpType.max,
                negate=True,
            )
            p = p_pool.tile([128, M], F32)
            zs = small.tile([128, 1], F32, tag="zs")
            nc.scalar.activation(
                p, lp, mybir.ActivationFunctionType.Exp,
                bias=nmx, scale=1.0, accum_out=zs,
            )
            rz = small.tile([128, 1], F32, tag="rz")
            nc.vector.reciprocal(rz, zs)
            nc.vector.tensor_scalar_mul(p, p, rz)

            # Two accumulation paths:
            #  - DVE: scalar_tensor_tensor FMA into acc_v (SBUF, fp32)
            #  - ACT: scaled copy z=p*y (bf16) + PE identity-matmul into
            #    pe_acc (PSUM accumulates in fp32)
            acc_v0 = acc_pool.tile([128, 128], F32, tag="accv0")
            acc_v1 = acc_pool.tile([128, 128], F32, tag="accv1")
            pe_acc = pa_ps.tile([128, 128], F32, tag="pacc")
            first_v0, first_v1, first_a = True, True, True
            n_act = NCHUNK * (4 - dve_per_chunk)
            act_done = 0

            for c in range(NCHUNK):
                yp = y_ps.tile([128, 512], F32, tag="yp")
                nc.tensor.matmul(
                    yp, xT, opsT_sb[:, c * 512:(c + 1) * 512],
                    start=True, stop=True,
                )
                for j in range(4):
                    m = 4 * c + j
                    ysl = yp[:, j * 128:(j + 1) * 128]
                    psl = p[:, m:m + 1]
                    use_dve = j < dve_per_chunk
                    if use_dve:
                        if j % 2 == 0:
                            acc_v, first_v = acc_v0, first_v0
                        else:
                            acc_v, first_v = acc_v1, first_v1
                        if first_v:
                            nc.vector.tensor_scalar_mul(acc_v, ysl, psl)
                            if j % 2 == 0:
                                first_v0 = False
                            else:
                                first_v1 = False
                        else:
                            nc.vector.scalar_tensor_tensor(
                                acc_v, ysl, psl, acc_v,
                                op0=mybir.AluOpType.mult,
                                op1=mybir.AluOpType.add,
                            )
                    else:
                        z = z_pool.tile([128, 128], mybir.dt.bfloat16, tag="z")
                        nc.scalar.mul(z, ysl, psl)
                        act_done += 1
                        nc.tensor.matmul(
                            pe_acc, ident_bf, z,
                            start=first_a, stop=(act_done == n_act),
                            skip_group_check=True,
                        )
                        first_a = False

            out_t = out_pool.tile([128, 128], F32)
            nc.vector.tensor_add(acc_v0, acc_v0, acc_v1)
            if n_act > 0:
                nc.vector.scalar_tensor_tensor(
                    out_t, pe_acc, 1.0, acc_v0,
                    op0=mybir.AluOpType.mult, op1=mybir.AluOpType.add,
                )
            else:
                nc.vector.tensor_copy(out_t, acc_v0)
            nc.sync.dma_start(out_d[i * 128:(i + 1) * 128, :], out_t)

    nc.compile()
    return nc


def _get_nc(**kw):
    key = tuple(sorted(kw.items()))
    if key not in _CACHE:
        _CACHE[key] = _build(**kw)
    return _CACHE[key]


def _run(inputs, trace=False, **build_kw):
    nc = _get_nc(**build_kw)
    x = np.ascontiguousarray(
        np.asarray(inputs["input_tensor"], np.float32).reshape(NTOK_TOTAL, D)
    )
    keys = np.ascontiguousarray(np.asarray(inputs["memory_keys"], np.float32))
    ops = np.ascontiguousarray(np.asarray(inputs["memory_ops"], np.float32))
    wq = np.ascontiguousarray(np.asarray(inputs["Wq"], np.float32))
    bq = np.ascontiguousarray(np.asarray(inputs["bq"], np.float32))

    in_maps = [
        {
            "x": x[c * NTOK:(c + 1) * NTOK],
            "keys": keys,
            "ops": ops,
            "wq": wq,
            "bq": bq,
        }
        for c in range(N_CORES)
    ]
    res = run_bass_kernel_spmd(
        nc, in_maps, core_ids=list(range(N_CORES)), trace=trace
    )
    out = np.concatenate([res.results[c]["out"] for c in range(N_CORES)], axis=0)
    return out.reshape(B, S, D), res


def kernel(**inputs) -> np.ndarray:
    out, _ = _run(inputs, trace=False)
    return out

